# revision 1
# baseline (speedup 1.0000x reference)
"""Graphormer layer (pre-norm MHSA + additive attn bias + SiLU FFN) on 8 trn2 cores.

Sharding: core c handles batch b = c//4 and query rows i0 = (c%4)*512 .. +512.
Each core computes LN1 + full K/V for its batch (replicated inside the
4-core batch group), Q/scores/softmax/attn@V for its 512 query rows, the
output projection, LN2 and the full FFN for those rows.  No collectives.

Host-side prep rotates each core's token axis by -i0 so the query block is
always columns 0:512 of the same SPMD program; the attn-bias j axis is
rotated identically (softmax/attn@V are order-invariant over j).

Layouts on device are feature-major ("transposed"): xT [D, T], qT/kT [d, T],
scoresT [j, i].  The softmax denominator comes from appending a ones column
to V in the attn@V matmul; normalization uses a gpsimd partition-broadcast
of the reciprocal.  Matmul operands are bf16 (fp32 accumulation in PSUM);
the residual path stays fp32.  Softmax skips the max-subtraction: scores
are O(8) here so exp stays comfortably inside fp32 range.
"""

import sys
from contextlib import ExitStack

import numpy as np

sys.path.insert(0, "/opt/trn_rl_repo")

import ml_dtypes  # noqa: E402

import concourse.bass as bass  # noqa: E402
import concourse.bacc as bacc  # noqa: E402
import concourse.tile as tile  # noqa: E402
from concourse import mybir  # noqa: E402
from concourse.bass_utils import run_bass_kernel_spmd  # noqa: E402

F32 = mybir.dt.float32
BF16 = mybir.dt.bfloat16
AF = mybir.ActivationFunctionType
OP = mybir.AluOpType
BF16_NP = ml_dtypes.bfloat16

B, T, D = 2, 2048, 1024
H, HD = 16, 64
FF = 4 * D
N_CORES = 8
IB = 512           # query rows per core
SCALE = 1.0 / 8.0  # 1/sqrt(HD)
EPS = 1e-5

_cache = {}


def build_program():
    nc = bacc.Bacc("TRN2", target_bir_lowering=False, debug=False)

    # ---- DRAM I/O ----
    xT_d = nc.dram_tensor("xT", [D, T], F32, kind="ExternalInput").ap()
    biasT_d = nc.dram_tensor("biasT", [H, T, IB], BF16, kind="ExternalInput").ap()
    Wq_d = nc.dram_tensor("Wq", [D, D], BF16, kind="ExternalInput").ap()
    Wk_d = nc.dram_tensor("Wk", [D, D], BF16, kind="ExternalInput").ap()
    Wv_d = nc.dram_tensor("Wv", [D, D], BF16, kind="ExternalInput").ap()
    Wo_d = nc.dram_tensor("Wo", [D, D], BF16, kind="ExternalInput").ap()
    W1_d = nc.dram_tensor("W1", [D, FF], BF16, kind="ExternalInput").ap()
    W2_d = nc.dram_tensor("W2", [FF, D], BF16, kind="ExternalInput").ap()
    # packed per-partition params: [128, n_tiles] fp32
    g1_d = nc.dram_tensor("g1", [128, 8], F32, kind="ExternalInput").ap()
    bg1_d = nc.dram_tensor("bg1", [128, 8], F32, kind="ExternalInput").ap()
    g2_d = nc.dram_tensor("g2", [128, 8], F32, kind="ExternalInput").ap()
    bg2_d = nc.dram_tensor("bg2", [128, 8], F32, kind="ExternalInput").ap()
    bq8_d = nc.dram_tensor("bq8", [128, 8], F32, kind="ExternalInput").ap()
    bk_d = nc.dram_tensor("bk", [128, 8], F32, kind="ExternalInput").ap()
    bo_d = nc.dram_tensor("bo", [128, 8], F32, kind="ExternalInput").ap()
    b1_d = nc.dram_tensor("b1", [128, 32], F32, kind="ExternalInput").ap()
    b2_d = nc.dram_tensor("b2", [128, 8], F32, kind="ExternalInput").ap()
    bv_d = nc.dram_tensor("bv", [1, D], BF16, kind="ExternalInput").ap()
    outT_d = nc.dram_tensor("outT", [D, IB], F32, kind="ExternalOutput").ap()

    with tile.TileContext(nc) as tc, ExitStack() as ctx:
        # ---------------- outermost (whole-kernel lifetime) ----------------
        const_p = ctx.enter_context(tc.tile_pool(name="const", bufs=1))
        param_p = ctx.enter_context(tc.tile_pool(name="param", bufs=1))
        res_p = ctx.enter_context(tc.tile_pool(name="res", bufs=1))
        oT_p = ctx.enter_context(tc.tile_pool(name="oT", bufs=1))
        out_p = ctx.enter_context(tc.tile_pool(name="out", bufs=2))

        ones_f = const_p.tile([128, 128], F32, tag="ones_f")
        nc.vector.memset(ones_f[:], 1.0)
        ones_b = const_p.tile([1, 128], BF16, tag="ones_b")
        nc.vector.memset(ones_b[:], 1.0)
        eps_t = const_p.tile([1, 1], F32, tag="eps")
        nc.vector.memset(eps_t[:], EPS)

        def load_param(name, dram, shape, dtype=F32):
            t = param_p.tile(shape, dtype, tag=name, name=name)
            nc.sync.dma_start(t[:], dram[:])
            return t

        g1 = load_param("g1", g1_d, [128, 8])
        bg1 = load_param("bg1", bg1_d, [128, 8])
        g2 = load_param("g2", g2_d, [128, 8])
        bg2 = load_param("bg2", bg2_d, [128, 8])
        bq8 = load_param("bq8", bq8_d, [128, 8])
        bk = load_param("bk", bk_d, [128, 8])
        bo = load_param("bo", bo_d, [128, 8])
        b1 = load_param("b1", b1_d, [128, 32])
        b2 = load_param("b2", b2_d, [128, 8])
        bv = load_param("bv", bv_d, [1, D], BF16)

        # res: x residual slice in phases A-D, then reused in place as
        # xres = x + attn_out for phases D-E.
        res = [res_p.tile([128, IB], F32, tag=f"res{e}", name=f"res{e}")
               for e in range(8)]
        oT = [oT_p.tile([128, IB], BF16, tag=f"oT{d}", name=f"oT{d}")
              for d in range(8)]

        # ---------------- scope: K/V/Q (phases A-C) ------------------------
        with tc.tile_pool(name="kT", bufs=1) as kT_p, \
             tc.tile_pool(name="vcat", bufs=1) as vcat_p, \
             tc.tile_pool(name="qT", bufs=1) as qT_p:
            kT = [kT_p.tile([128, T], BF16, tag=f"kT{d}", name=f"kT{d}")
                  for d in range(8)]
            vcat = [vcat_p.tile([128, H * (HD + 1)], BF16, tag=f"vc{t}",
                                name=f"vc{t}") for t in range(16)]
            qT = [qT_p.tile([128, IB], BF16, tag=f"qT{d}", name=f"qT{d}")
                  for d in range(8)]

            # ---------------- scope: hT (phases A-B) -----------------------
            with tc.tile_pool(name="hT", bufs=1) as hT_p:
                hT = [hT_p.tile([128, T], BF16, tag=f"hT{e}", name=f"hT{e}")
                      for e in range(8)]

                # ===== Phase A: LN1 (streamed, partition-axis stats) =======
                with tc.tile_pool(name="xc", bufs=2) as xc_p, \
                     tc.tile_pool(name="sq", bufs=3) as sq_p, \
                     tc.tile_pool(name="lnt", bufs=2) as lnt_p, \
                     tc.tile_pool(name="lnb", bufs=2) as lnb_p, \
                     tc.tile_pool(name="lnps", bufs=2,
                                  space=bass.MemorySpace.PSUM) as lnps_p:
                    for n in range(4):
                        nb = slice(n * 512, (n + 1) * 512)
                        xcs = []
                        ps_mu = lnps_p.tile([1, 512], F32, tag="psmu", name="psmu")
                        ps_sq = lnps_p.tile([1, 512], F32, tag="pssq", name="pssq")
                        for e in range(8):
                            xc = xc_p.tile([128, 512], F32, tag=f"xc{e}", name="xc")
                            nc.sync.dma_start(xc[:], xT_d[e * 128:(e + 1) * 128, nb])
                            xcs.append(xc)
                            nc.tensor.matmul(ps_mu[:], ones_f[:, 0:1], xc[:],
                                             start=(e == 0), stop=(e == 7))
                            x2 = sq_p.tile([128, 512], F32, tag="x2", name="x2")
                            nc.scalar.square(x2[:], xc[:])
                            nc.tensor.matmul(ps_sq[:], ones_f[:, 0:1], x2[:],
                                             start=(e == 0), stop=(e == 7))
                        mu_n = lnt_p.tile([1, 512], F32, tag="mu_n", name="mu_n")
                        nc.scalar.activation(mu_n[:], ps_mu[:], AF.Identity,
                                             scale=1.0 / D)
                        mu2_n = lnt_p.tile([1, 512], F32, tag="mu2_n", name="mu2_n")
                        nc.scalar.square(mu2_n[:], mu_n[:])
                        var_n = lnt_p.tile([1, 512], F32, tag="var_n", name="var_n")
                        nc.vector.scalar_tensor_tensor(
                            var_n[:], ps_sq[:], 1.0 / D, mu2_n[:],
                            op0=OP.mult, op1=OP.subtract)
                        std_n = lnt_p.tile([1, 512], F32, tag="std_n", name="std_n")
                        nc.scalar.activation(std_n[:], var_n[:], AF.Sqrt, bias=eps_t[:])
                        rstd_n = lnt_p.tile([1, 512], F32, tag="rstd_n", name="rstd_n")
                        nc.vector.reciprocal(rstd_n[:], std_n[:])
                        mu_b = lnb_p.tile([128, 512], F32, tag="mu_b", name="mu_b")
                        nc.gpsimd.partition_broadcast(mu_b[:], mu_n[:])
                        rstd_b = lnb_p.tile([128, 512], F32, tag="rstd_b",
                                            name="rstd_b")
                        nc.gpsimd.partition_broadcast(rstd_b[:], rstd_n[:])
                        for e in range(8):
                            if n == 0:
                                nc.scalar.activation(res[e][:], xcs[e][:],
                                                     AF.Identity)
                            t = sq_p.tile([128, 512], F32, tag="lnap", name="lnap")
                            nc.vector.tensor_sub(t[:], xcs[e][:], mu_b[:])
                            nc.vector.tensor_mul(t[:], t[:], rstd_b[:])
                            nc.scalar.activation(hT[e][:, nb], t[:], AF.Identity,
                                                 scale=g1[:, e:e + 1],
                                                 bias=bg1[:, e:e + 1])

                # ===== Phase B: Q/K/V projections ==========================
                with tc.tile_pool(name="wp", bufs=12) as wp, \
                     tc.tile_pool(name="wv512", bufs=2) as wv_p, \
                     tc.tile_pool(name="pps", bufs=4,
                                  space=bass.MemorySpace.PSUM) as pps:
                    # qT[d, i] for this core's rows (= token cols 0:IB)
                    for dt in range(8):
                        ps = pps.tile([128, 512], F32, tag="ps", name="psq")
                        for e in range(8):
                            wt = wp.tile([128, 128], BF16, tag="w", name="wq")
                            nc.sync.dma_start(
                                wt[:], Wq_d[e * 128:(e + 1) * 128,
                                            dt * 128:(dt + 1) * 128])
                            nc.tensor.matmul(ps[:], wt[:], hT[e][:, 0:IB],
                                             start=(e == 0), stop=(e == 7))
                        nc.scalar.activation(qT[dt][:], ps[:], AF.Identity,
                                             scale=SCALE, bias=bq8[:, dt:dt + 1])
                    # kT[d, j] over all tokens
                    for dt in range(8):
                        for n in range(4):
                            nb = slice(n * 512, (n + 1) * 512)
                            ps = pps.tile([128, 512], F32, tag="ps", name="psk")
                            for e in range(8):
                                wt = wp.tile([128, 128], BF16, tag="w", name="wk")
                                nc.sync.dma_start(
                                    wt[:], Wk_d[e * 128:(e + 1) * 128,
                                                dt * 128:(dt + 1) * 128])
                                nc.tensor.matmul(ps[:], wt[:], hT[e][:, nb],
                                                 start=(e == 0), stop=(e == 7))
                            nc.scalar.activation(kT[dt][:, nb], ps[:], AF.Identity,
                                                 bias=bk[:, dt:dt + 1])
                    # v[j, d] natural layout + ones column per head
                    for tt in range(16):
                        nc.vector.memset(
                            vcat[tt][:].rearrange(
                                "p (h x) -> p h x", x=HD + 1)[:, :, HD:HD + 1],
                            1.0)
                    for n in range(2):
                        nb = slice(n * 512, (n + 1) * 512)
                        wv_tiles = []
                        for e in range(8):
                            wv = wv_p.tile([128, 512], BF16, tag=f"wv{e}",
                                           name=f"wv{e}")
                            nc.sync.dma_start(wv[:], Wv_d[e * 128:(e + 1) * 128, nb])
                            wv_tiles.append(wv)
                        for tt in range(16):
                            tb = slice(tt * 128, (tt + 1) * 128)
                            ps = pps.tile([128, 512], F32, tag="ps", name="psv")
                            for e in range(8):
                                nc.tensor.matmul(ps[:], hT[e][:, tb],
                                                 wv_tiles[e][:],
                                                 start=(e == 0), stop=False)
                            nc.tensor.matmul(ps[:], ones_b[:], bv[:, nb],
                                             start=False, stop=True)
                            dst = vcat[tt][:, n * 8 * (HD + 1):(n + 1) * 8 * (HD + 1)]
                            dst = dst.rearrange("p (h x) -> p h x",
                                                x=HD + 1)[:, :, 0:HD]
                            src = ps[:].rearrange("p (h d) -> p h d", d=HD)
                            nc.scalar.activation(dst, src, AF.Identity)
            # hT pool closed here

            # ===== Phase C: attention ======================================
            with tc.tile_pool(name="biasdma", bufs=8) as bias_p, \
                 tc.tile_pool(name="upre", bufs=4) as up_p, \
                 tc.tile_pool(name="uexp", bufs=4) as u_p, \
                 tc.tile_pool(name="nrm", bufs=2) as nrm_p, \
                 tc.tile_pool(name="pss", bufs=2,
                              space=bass.MemorySpace.PSUM) as pss, \
                 tc.tile_pool(name="pso", bufs=2,
                              space=bass.MemorySpace.PSUM) as pso:
                for h in range(H):
                    dt, po = h // 2, (h % 2) * 64
                    ps_o = pso.tile([HD + 1, 512], F32, tag="ps_o", name="ps_o")
                    for j in range(16):
                        jb = slice(j * 128, (j + 1) * 128)
                        ps_s = pss.tile([128, 512], F32, tag="ps_s", name="ps_s")
                        nc.tensor.matmul(ps_s[:], kT[dt][po:po + 64, jb],
                                         qT[dt][po:po + 64, :],
                                         start=True, stop=True)
                        bt = bias_p.tile([128, IB], BF16, tag="bt", name="bt")
                        nc.sync.dma_start(bt[:], biasT_d[h, jb, :])
                        up = up_p.tile([128, IB], F32, tag="up", name="up")
                        nc.vector.scalar_tensor_tensor(up[:], ps_s[:], 1.0, bt[:],
                                                       op0=OP.mult, op1=OP.add)
                        u = u_p.tile([128, IB], BF16, tag="u", name="u")
                        nc.scalar.activation(u[:], up[:], AF.Exp)
                        nc.tensor.matmul(
                            ps_o[:], vcat[j][:, h * (HD + 1):(h + 1) * (HD + 1)],
                            u[:], start=(j == 0), stop=(j == 15))
                    recip = nrm_p.tile([1, 512], F32, tag="recip", name="recip")
                    nc.vector.reciprocal(recip[:], ps_o[64:65, :])
                    rb = nrm_p.tile([64, 512], F32, tag="rb", name="rb")
                    nc.gpsimd.partition_broadcast(rb[:], recip[:])
                    nc.vector.tensor_mul(oT[dt][po:po + 64, :], ps_o[0:64, :],
                                         rb[:])
        # kT/vcat/qT pools closed here

        # ---------------- scope: h2/sz (phases D-E) ------------------------
        with tc.tile_pool(name="h2", bufs=1) as h2_p, \
             tc.tile_pool(name="sz", bufs=1) as sz_p:
            h2 = [h2_p.tile([128, IB], BF16, tag=f"h2{e}", name=f"h2{e}")
                  for e in range(8)]
            sz = [sz_p.tile([128, IB], BF16, tag=f"sz{f}", name=f"sz{f}")
                  for f in range(32)]

            # ===== Phase D: out-projection + LN2 ===========================
            with tc.tile_pool(name="wp2", bufs=12) as wp2, \
                 tc.tile_pool(name="sq2", bufs=3) as sq2_p, \
                 tc.tile_pool(name="lnt2", bufs=2) as lnt2_p, \
                 tc.tile_pool(name="lnb2", bufs=2) as lnb2_p, \
                 tc.tile_pool(name="dps", bufs=2,
                              space=bass.MemorySpace.PSUM) as dps, \
                 tc.tile_pool(name="dps1", bufs=1,
                              space=bass.MemorySpace.PSUM) as dps1:
                for et in range(8):
                    ps = dps.tile([128, 512], F32, tag="psx1", name="psx1")
                    for dt in range(8):
                        wt = wp2.tile([128, 128], BF16, tag="w2", name="wo")
                        nc.sync.dma_start(wt[:], Wo_d[dt * 128:(dt + 1) * 128,
                                                      et * 128:(et + 1) * 128])
                        nc.tensor.matmul(ps[:], wt[:], oT[dt][:],
                                         start=(dt == 0), stop=(dt == 7))
                    # res[et] <- x + attn_out (+bo), in place
                    nc.vector.scalar_tensor_tensor(res[et][:], ps[:],
                                                   bo[:, et:et + 1], res[et][:],
                                                   op0=OP.add, op1=OP.add)
                # LN2 (single 512-col block)
                ps_mu = dps1.tile([1, 512], F32, tag="psmu2", name="psmu2")
                for e in range(8):
                    nc.tensor.matmul(ps_mu[:], ones_f[:, 0:1], res[e][:],
                                     start=(e == 0), stop=(e == 7))
                ps_sq = dps1.tile([1, 512], F32, tag="pssq2", name="pssq2")
                for e in range(8):
                    x2 = sq2_p.tile([128, 512], F32, tag="x22", name="x22")
                    nc.scalar.square(x2[:], res[e][:])
                    nc.tensor.matmul(ps_sq[:], ones_f[:, 0:1], x2[:],
                                     start=(e == 0), stop=(e == 7))
                mu_n = lnt2_p.tile([1, 512], F32, tag="mu_n2", name="mu_n2")
                nc.scalar.activation(mu_n[:], ps_mu[:], AF.Identity, scale=1.0 / D)
                mu2_n = lnt2_p.tile([1, 512], F32, tag="mu2_n2", name="mu2_n2")
                nc.scalar.square(mu2_n[:], mu_n[:])
                var_n = lnt2_p.tile([1, 512], F32, tag="var_n2", name="var_n2")
                nc.vector.scalar_tensor_tensor(var_n[:], ps_sq[:], 1.0 / D,
                                               mu2_n[:], op0=OP.mult,
                                               op1=OP.subtract)
                std_n = lnt2_p.tile([1, 512], F32, tag="std_n2", name="std_n2")
                nc.scalar.activation(std_n[:], var_n[:], AF.Sqrt, bias=eps_t[:])
                rstd_n = lnt2_p.tile([1, 512], F32, tag="rstd_n2", name="rstd_n2")
                nc.vector.reciprocal(rstd_n[:], std_n[:])
                mu_b = lnb2_p.tile([128, 512], F32, tag="mu_b2", name="mu_b2")
                nc.gpsimd.partition_broadcast(mu_b[:], mu_n[:])
                rstd_b = lnb2_p.tile([128, 512], F32, tag="rstd_b2",
                                     name="rstd_b2")
                nc.gpsimd.partition_broadcast(rstd_b[:], rstd_n[:])
                for e in range(8):
                    t = sq2_p.tile([128, IB], F32, tag="lnap2", name="lnap2")
                    nc.vector.tensor_sub(t[:], res[e][:], mu_b[:])
                    nc.vector.tensor_mul(t[:], t[:], rstd_b[:])
                    nc.scalar.activation(h2[e][:], t[:], AF.Identity,
                                         scale=g2[:, e:e + 1],
                                         bias=bg2[:, e:e + 1])

            # ===== Phase E: FFN ============================================
            with tc.tile_pool(name="wp3", bufs=16) as wp3, \
                 tc.tile_pool(name="sg", bufs=3) as sg_p, \
                 tc.tile_pool(name="eps", bufs=4,
                              space=bass.MemorySpace.PSUM) as eps_p:
                for ft in range(32):
                    ps = eps_p.tile([128, 512], F32, tag="pse", name="psz")
                    for e in range(8):
                        wt = wp3.tile([128, 128], BF16, tag="w3", name="w1t")
                        nc.sync.dma_start(wt[:], W1_d[e * 128:(e + 1) * 128,
                                                      ft * 128:(ft + 1) * 128])
                        nc.tensor.matmul(ps[:], wt[:], h2[e][:],
                                         start=(e == 0), stop=(e == 7))
                    sg = sg_p.tile([128, IB], BF16, tag="sg", name="sg")
                    nc.scalar.activation(sg[:], ps[:], AF.Sigmoid,
                                         bias=b1[:, ft:ft + 1])
                    # silu(z) = z * sigmoid(z), z = ps + b1
                    nc.vector.scalar_tensor_tensor(sz[ft][:], ps[:],
                                                   b1[:, ft:ft + 1], sg[:],
                                                   op0=OP.add, op1=OP.mult)
                for et in range(8):
                    ps = eps_p.tile([128, 512], F32, tag="pse", name="psy")
                    for ft in range(32):
                        wt = wp3.tile([128, 128], BF16, tag="w3", name="w2t")
                        nc.sync.dma_start(wt[:], W2_d[ft * 128:(ft + 1) * 128,
                                                      et * 128:(et + 1) * 128])
                        nc.tensor.matmul(ps[:], wt[:], sz[ft][:],
                                         start=(ft == 0), stop=(ft == 31))
                    ot = out_p.tile([128, IB], F32, tag="outt", name="outt")
                    nc.vector.scalar_tensor_tensor(ot[:], ps[:], b2[:, et:et + 1],
                                                   res[et][:], op0=OP.add,
                                                   op1=OP.add)
                    nc.sync.dma_start(outT_d[et * 128:(et + 1) * 128, :], ot[:])

    nc.compile()
    return nc


def _prep_inputs(inputs):
    """Host-side layout prep -> list of 8 per-core input maps."""
    x = np.asarray(inputs["x"], dtype=np.float32)
    ab = np.asarray(inputs["attn_bias"], dtype=np.float32)

    def pack(v, ntiles):
        return np.ascontiguousarray(
            np.asarray(v, np.float32).reshape(ntiles, 128).T)

    shared = {
        "Wq": np.ascontiguousarray(np.asarray(inputs["Wq"]).astype(BF16_NP)),
        "Wk": np.ascontiguousarray(np.asarray(inputs["Wk"]).astype(BF16_NP)),
        "Wv": np.ascontiguousarray(np.asarray(inputs["Wv"]).astype(BF16_NP)),
        "Wo": np.ascontiguousarray(np.asarray(inputs["Wo"]).astype(BF16_NP)),
        "W1": np.ascontiguousarray(np.asarray(inputs["W1"]).astype(BF16_NP)),
        "W2": np.ascontiguousarray(np.asarray(inputs["W2"]).astype(BF16_NP)),
        "g1": pack(inputs["ln1_g"], 8),
        "bg1": pack(inputs["ln1_b"], 8),
        "g2": pack(inputs["ln2_g"], 8),
        "bg2": pack(inputs["ln2_b"], 8),
        "bq8": pack(np.asarray(inputs["bq"], np.float32) * SCALE, 8),
        "bk": pack(inputs["bk"], 8),
        "bo": pack(inputs["bo"], 8),
        "b1": pack(inputs["b1"], 32),
        "b2": pack(inputs["b2"], 8),
        "bv": np.ascontiguousarray(
            np.asarray(inputs["bv"], np.float32).astype(BF16_NP).reshape(1, D)),
    }
    xT = [np.ascontiguousarray(x[b].T) for b in range(B)]  # [D, T] f32
    ab_bf = ab.astype(BF16_NP)  # [B, H, T(i), T(j)]
    in_maps = []
    for c in range(N_CORES):
        b, i0 = c // 4, (c % 4) * IB
        # token axis rotated by -i0 (queries land at cols 0:IB); the j axis
        # of the bias is rotated identically to match k/v token order.
        xTc = np.ascontiguousarray(np.roll(xT[b], -i0, axis=1))
        biasT = np.ascontiguousarray(
            np.roll(ab_bf[b, :, i0:i0 + IB, :], -i0, axis=2).transpose(0, 2, 1))
        m = {"xT": xTc, "biasT": biasT}
        m.update(shared)
        in_maps.append(m)
    return in_maps


def kernel(**inputs):
    if "nc" not in _cache:
        _cache["nc"] = build_program()
    nc = _cache["nc"]
    in_maps = _prep_inputs(inputs)
    r = run_bass_kernel_spmd(nc, in_maps, list(range(N_CORES)))
    out = np.empty((B, T, D), dtype=np.float32)
    for c in range(N_CORES):
        b, i0 = c // 4, (c % 4) * IB
        out[b, i0:i0 + IB, :] = np.asarray(r.results[c]["outT"], np.float32).T
    return out



# revision 2
# speedup vs baseline: 11.2451x; 11.2451x over previous
"""Graphormer layer (pre-norm MHSA + additive attn bias + SiLU FFN) on 8 trn2 cores.

Sharding: core c handles batch b = c//4 and query rows i0 = (c%4)*512 .. +512.
Each core computes LN1 + full K/V for its batch (replicated inside the
4-core batch group), Q/scores/softmax/attn@V for its 512 query rows, the
output projection, LN2 and the full FFN for those rows.  No collectives.

Host-side prep rotates each core's token axis by -i0 so the query block is
always columns 0:512 of the same SPMD program; the attn-bias j axis is
rotated identically (softmax/attn@V are order-invariant over j).

v2 vs baseline:
- Coalesced DMAs (~40 large transfers instead of ~1200 32KB tiles), split
  across both HWDGE rings: nc.sync carries x/bias/out, nc.scalar carries
  weights/params.
- LN affine (g,b) folded into the downstream projection weights/biases on
  the host; LN normalize is 2 bf16 DVE ops in place of the x tile.
- attn softmax: exp(attn_bias) precomputed on host; device does
  exp(scores) on ACT straight from PSUM, then one bf16 DVE multiply.
  Denominator via the ones-column in the packed V tile.
- Scores matmuls row-packed: the two 64-contraction heads of a kT tile
  run concurrently in the PE array (tile_position row groups 0/64).
- FFN1 epilogue is a single fused ACT Silu; FFN2 runs ft-outer with 8
  held PSUM accumulation groups so W2 streams in halves.
"""

import sys
from contextlib import ExitStack

import numpy as np

sys.path.insert(0, "/opt/trn_rl_repo")

import ml_dtypes  # noqa: E402

import concourse.bass as bass  # noqa: E402
import concourse.bacc as bacc  # noqa: E402
import concourse.tile as tile  # noqa: E402
from concourse import mybir  # noqa: E402
from concourse.bass_utils import run_bass_kernel_spmd  # noqa: E402

F32 = mybir.dt.float32
BF16 = mybir.dt.bfloat16
AF = mybir.ActivationFunctionType
OP = mybir.AluOpType
BF16_NP = ml_dtypes.bfloat16

B, T, D = 2, 2048, 1024
H, HD = 16, 64
FF = 4 * D
N_CORES = 8
IB = 512           # query rows per core
SCALE = 1.0 / 8.0  # 1/sqrt(HD)
EPS = 1e-5

_cache = {}


def build_program():
    nc = bacc.Bacc("TRN2", target_bir_lowering=False, debug=False)

    # ---- DRAM I/O ----
    xb_d = nc.dram_tensor("xb", [D, T], BF16, kind="ExternalInput").ap()
    xr_d = nc.dram_tensor("xr", [D, IB], F32, kind="ExternalInput").ap()
    eb_d = nc.dram_tensor("eb", [H, T, IB], BF16, kind="ExternalInput").ap()
    wq_d = nc.dram_tensor("wq", [D, D], BF16, kind="ExternalInput").ap()
    wk_d = nc.dram_tensor("wk", [D, D], BF16, kind="ExternalInput").ap()
    wv_d = nc.dram_tensor("wv", [D, D], BF16, kind="ExternalInput").ap()
    wo_d = nc.dram_tensor("wo", [D, D], BF16, kind="ExternalInput").ap()
    w1_d = nc.dram_tensor("w1", [D, FF], BF16, kind="ExternalInput").ap()
    w2_d = nc.dram_tensor("w2", [FF, D], BF16, kind="ExternalInput").ap()
    bqc_d = nc.dram_tensor("bqc", [128, 8], F32, kind="ExternalInput").ap()
    bkc_d = nc.dram_tensor("bkc", [128, 8], F32, kind="ExternalInput").ap()
    boc_d = nc.dram_tensor("boc", [128, 8], F32, kind="ExternalInput").ap()
    b1c_d = nc.dram_tensor("b1c", [128, 32], F32, kind="ExternalInput").ap()
    b2c_d = nc.dram_tensor("b2c", [128, 8], F32, kind="ExternalInput").ap()
    bvc_d = nc.dram_tensor("bvc", [1, D], BF16, kind="ExternalInput").ap()
    outT_d = nc.dram_tensor("outT", [D, IB], F32, kind="ExternalOutput").ap()

    with tile.TileContext(nc) as tc, ExitStack() as ctx:
        # ---------------- whole-kernel pools ----------------
        const_p = ctx.enter_context(tc.tile_pool(name="const", bufs=1))
        param_p = ctx.enter_context(tc.tile_pool(name="param", bufs=1))
        ps_p = ctx.enter_context(
            tc.tile_pool(name="ps", bufs=8, space=bass.MemorySpace.PSUM))
        oT_p = ctx.enter_context(tc.tile_pool(name="oT", bufs=1))
        wo_p = ctx.enter_context(tc.tile_pool(name="wo", bufs=1))
        w1_p = ctx.enter_context(tc.tile_pool(name="w1", bufs=2))

        ones_b16 = const_p.tile([128, 1], BF16, tag="ones_b16")
        nc.vector.memset(ones_b16[:], 1.0)
        onesr_b = const_p.tile([1, 128], BF16, tag="onesr_b")
        nc.vector.memset(onesr_b[:], 1.0)
        eps_t = const_p.tile([1, 1], F32, tag="eps")
        nc.vector.memset(eps_t[:], EPS)

        def load_param(name, dram, shape, dtype=F32):
            t = param_p.tile(shape, dtype, tag=name, name=name)
            nc.scalar.dma_start(t[:], dram[:])
            return t

        bqc = load_param("bqc", bqc_d, [128, 8])
        bkc = load_param("bkc", bkc_d, [128, 8])
        boc = load_param("boc", boc_d, [128, 8])
        b1c = load_param("b1c", b1c_d, [128, 32])
        b2c = load_param("b2c", b2c_d, [128, 8])
        bvc = load_param("bvc", bvc_d, [1, D], BF16)

        # oT: attention output, feature-major bf16.
        oT = oT_p.tile([128, 8 * IB], BF16, tag="oT", name="oT")
        # wo/w1 tiles are allocated up front (fresh addresses, no zone
        # deps); their DMAs are traced in phase B after wq/wk/wv so the
        # ACT HWDGE ring serves the phase-B weights first.
        wo = wo_p.tile([128, 8 * D], BF16, tag="wo", name="wo")
        w1h = [w1_p.tile([128, 8 * 1024], BF16, tag="w1", name="w1")
               for _ in range(4)]

        # ---------------- scope: phases A-C ----------------
        with tc.tile_pool(name="kT", bufs=1) as kT_p, \
             tc.tile_pool(name="vc", bufs=1) as vc_p, \
             tc.tile_pool(name="qT", bufs=1) as qT_p:
            kT = kT_p.tile([128, 8 * T], BF16, tag="kT", name="kT")
            vcat = vc_p.tile([128, 16 * H * (HD + 1)], BF16, tag="vc", name="vc")
            qT = qT_p.tile([128, 8 * IB], BF16, tag="qT", name="qT")

            # ones column per (j-tile, head) for the softmax denominator
            nc.vector.memset(
                vcat[:].rearrange("p (t h x) -> p t h x", h=H, x=HD + 1)
                [:, :, :, HD:HD + 1], 1.0)

            with tc.tile_pool(name="xbh", bufs=1) as xbh_p:
                # xb doubles as hT: LN1 normalizes it in place (bf16).
                xb = xbh_p.tile([128, 8 * T], BF16, tag="xbh", name="xbh")
                for n in range(4):
                    nc.sync.dma_start(
                        xb[:].rearrange("p (e t) -> p e t", e=8)
                        [:, :, n * 512:(n + 1) * 512],
                        xb_d.rearrange("(e p) t -> p e t", p=128)
                        [:, :, n * 512:(n + 1) * 512])

                # ===== Phase A: LN1 stats + in-place normalize ==========
                with tc.tile_pool(name="sq", bufs=3) as sq_p, \
                     tc.tile_pool(name="lnr", bufs=2) as lnr_p, \
                     tc.tile_pool(name="lnb", bufs=2) as lnb_p:
                    for n in range(4):
                        nb = slice(n * 512, (n + 1) * 512)
                        ps_mu = ps_p.tile([1, 512], F32, tag="ps", name="psmu")
                        ps_sq = ps_p.tile([1, 512], F32, tag="ps", name="pssq")
                        for e in range(8):
                            xsl = xb[:, e * T + n * 512: e * T + (n + 1) * 512]
                            nc.tensor.matmul(ps_mu[:], ones_b16[:], xsl,
                                             start=(e == 0), stop=(e == 7))
                            x2 = sq_p.tile([128, 512], BF16, tag="x2", name="x2")
                            nc.scalar.square(x2[:], xsl)
                            nc.tensor.matmul(ps_sq[:], ones_b16[:], x2[:],
                                             start=(e == 0), stop=(e == 7))
                        mu = lnr_p.tile([1, 512], F32, tag="mu", name="mu")
                        nc.scalar.activation(mu[:], ps_mu[:], AF.Identity,
                                             scale=1.0 / D)
                        mu2 = lnr_p.tile([1, 512], F32, tag="mu2", name="mu2")
                        nc.scalar.square(mu2[:], mu[:])
                        var = lnr_p.tile([1, 512], F32, tag="var", name="var")
                        nc.vector.scalar_tensor_tensor(
                            var[:], ps_sq[:], 1.0 / D, mu2[:],
                            op0=OP.mult, op1=OP.subtract)
                        std = lnr_p.tile([1, 512], F32, tag="std", name="std")
                        nc.scalar.activation(std[:], var[:], AF.Sqrt,
                                             bias=eps_t[:])
                        rstd = lnr_p.tile([1, 512], F32, tag="rstd", name="rstd")
                        nc.vector.reciprocal(rstd[:], std[:])
                        mu_h = lnr_p.tile([1, 512], BF16, tag="mu_h", name="mu_h")
                        nc.scalar.activation(mu_h[:], mu[:], AF.Copy)
                        rstd_h = lnr_p.tile([1, 512], BF16, tag="rstd_h",
                                            name="rstd_h")
                        nc.scalar.activation(rstd_h[:], rstd[:], AF.Copy)
                        mu_b = lnb_p.tile([128, 512], BF16, tag="mu_b",
                                          name="mu_b")
                        nc.gpsimd.partition_broadcast(mu_b[:], mu_h[:])
                        rstd_b = lnb_p.tile([128, 512], BF16, tag="rstd_b",
                                            name="rstd_b")
                        nc.gpsimd.partition_broadcast(rstd_b[:], rstd_h[:])
                        for e in range(8):
                            hsl = xb[:, e * T + n * 512: e * T + (n + 1) * 512]
                            nc.vector.tensor_sub(hsl, hsl, mu_b[:])
                            nc.vector.tensor_mul(hsl, hsl, rstd_b[:])

                # ===== Phase B: K/V then Q projections ==================
                with tc.tile_pool(name="wk", bufs=1) as wk_p, \
                     tc.tile_pool(name="wv", bufs=1) as wv_p:
                    # All no-dep weight loads issue at the top of the ACT
                    # ring, in this order, and transfer during phase A.
                    wk = wk_p.tile([128, 8 * D], BF16, tag="wk", name="wk")
                    wv = wv_p.tile([128, 8 * D], BF16, tag="wv", name="wv")
                    with tc.high_priority():
                        nc.scalar.dma_start(
                            wk[:].rearrange("p (e d) -> p e d", e=8),
                            wk_d.rearrange("(e p) d -> p e d", p=128))
                        nc.sync.dma_start(
                            wv[:].rearrange("p (e d) -> p e d", e=8),
                            wv_d.rearrange("(e p) d -> p e d", p=128))
                    # kT[d, j]: token-chunk outer so each group only needs
                    # one normalized chunk (overlaps the LN1 tail).
                    for n in range(4):
                        for dt in range(8):
                            pk = ps_p.tile([128, 512], F32, tag="ps",
                                           name="psk")
                            for e in range(8):
                                nc.tensor.matmul(
                                    pk[:],
                                    wk[:, e * D + dt * 128:
                                       e * D + (dt + 1) * 128],
                                    xb[:, e * T + n * 512: e * T + (n + 1) * 512],
                                    start=(e == 0), stop=(e == 7))
                            with tc.high_priority():
                                nc.scalar.activation(
                                    kT[:, dt * T + n * 512:
                                       dt * T + (n + 1) * 512],
                                    pk[:], AF.Identity, bias=bkc[:, dt:dt + 1])
                    # v[j, d] natural layout into vcat (+ ones columns)
                    for n2 in range(2):
                        for tt in range(16):
                            ps_v = ps_p.tile([128, 512], F32, tag="ps",
                                             name="psv")
                            for e in range(8):
                                nc.tensor.matmul(
                                    ps_v[:],
                                    xb[:, e * T + tt * 128: e * T + (tt + 1) * 128],
                                    wv[:, e * D + n2 * 512: e * D + (n2 + 1) * 512],
                                    start=(e == 0), stop=False)
                            nc.tensor.matmul(ps_v[:], onesr_b[:],
                                             bvc[:, n2 * 512:(n2 + 1) * 512],
                                             start=False, stop=True)
                            dst = vcat[:, tt * H * (HD + 1) + 8 * n2 * (HD + 1):
                                       tt * H * (HD + 1) + (8 * n2 + 8) * (HD + 1)]
                            dst = dst.rearrange("p (h x) -> p h x",
                                                x=HD + 1)[:, :, 0:HD]
                            src = ps_v[:].rearrange("p (h d) -> p h d", d=HD)
                            with tc.high_priority():
                                nc.scalar.activation(dst, src, AF.Identity)

                # Q last: wq streams in while K/V compute runs.
                with tc.tile_pool(name="wq", bufs=1) as wq_p:
                    wq = wq_p.tile([128, 8 * D], BF16, tag="wq", name="wq")
                    nc.sync.dma_start(
                        wq[:].rearrange("p (e d) -> p e d", e=8),
                        wq_d.rearrange("(e p) d -> p e d", p=128))
                    for dt in range(8):
                        ps_q = ps_p.tile([128, IB], F32, tag="ps", name="psq")
                        for e in range(8):
                            nc.tensor.matmul(
                                ps_q[:],
                                wq[:, e * D + dt * 128: e * D + (dt + 1) * 128],
                                xb[:, e * T: e * T + IB],
                                start=(e == 0), stop=(e == 7))
                        with tc.high_priority():
                            nc.scalar.activation(qT[:, dt * IB:(dt + 1) * IB],
                                                 ps_q[:], AF.Identity,
                                                 bias=bqc[:, dt:dt + 1])
            # xbh (hT) freed here

            # ===== Phase C: attention ===================================
            with tc.tile_pool(name="eb", bufs=7) as eb_p, \
                 tc.tile_pool(name="u", bufs=4) as u_p, \
                 tc.tile_pool(name="nrm", bufs=2) as nrm_p:
                for hp in range(8):
                    dt = hp
                    ebt = []
                    for hh in range(2):
                        h = 2 * hp + hh
                        halves = []
                        for jh in range(2):
                            t_eb = eb_p.tile([128, 8 * IB], BF16, tag="eb",
                                             name="eb")
                            nc.sync.dma_start(
                                t_eb[:].rearrange("p (j i) -> p j i", j=8),
                                eb_d[h].rearrange("(j p) i -> p j i", p=128)
                                [:, jh * 8:(jh + 1) * 8, :])
                            halves.append(t_eb)
                        ebt.append(halves)
                    if hp == 1:
                        nc.sync.dma_start(
                            wo[:].rearrange("p (d e) -> p d e", d=8),
                            wo_d.rearrange("(d p) e -> p d e", p=128))
                    elif 2 <= hp <= 5:
                        k = hp - 2
                        nc.sync.dma_start(
                            w1h[k][:].rearrange("p (e f) -> p e f", e=8),
                            w1_d[:, k * 1024:(k + 1) * 1024]
                            .rearrange("(e p) f -> p e f", p=128))
                    ps_o = [ps_p.tile([HD + 1, IB], F32, tag="ps", name="pso")
                            for _ in range(2)]
                    for j in range(16):
                        for hh in range(2):
                            h = 2 * hp + hh
                            po = hh * 64
                            ps_s = ps_p.tile([128, IB], F32, tag="ps",
                                             name="pss")
                            nc.tensor.matmul(
                                ps_s[:],
                                kT[po:po + 64, dt * T + j * 128:
                                   dt * T + (j + 1) * 128],
                                qT[po:po + 64, dt * IB:(dt + 1) * IB],
                                start=True, stop=True)
                            u = u_p.tile([128, IB], BF16, tag="u", name="u")
                            nc.scalar.activation(u[:], ps_s[:], AF.Exp)
                            nc.vector.tensor_mul(
                                u[:], u[:],
                                ebt[hh][j // 8][:, (j % 8) * IB:
                                                (j % 8 + 1) * IB])
                            nc.tensor.matmul(
                                ps_o[hh][:],
                                vcat[:, j * H * (HD + 1) + h * (HD + 1):
                                     j * H * (HD + 1) + (h + 1) * (HD + 1)],
                                u[:], start=(j == 0), stop=(j == 15))
                    for hh in range(2):
                        po = hh * 64
                        rec = nrm_p.tile([1, IB], F32, tag="rec", name="rec")
                        nc.vector.reciprocal(rec[:], ps_o[hh][HD:HD + 1, :])
                        rb = nrm_p.tile([64, IB], F32, tag="rb", name="rb")
                        nc.gpsimd.partition_broadcast(rb[:], rec[:])
                        nc.vector.tensor_mul(
                            oT[po:po + 64, dt * IB:(dt + 1) * IB],
                            ps_o[hh][0:HD, :], rb[:])
        # eb/kT/vcat/qT freed here

        # ---------------- scope: phases D-E ----------------
        with tc.tile_pool(name="res", bufs=1) as res_p, \
             tc.tile_pool(name="rb16", bufs=1) as rb16_p, \
             tc.tile_pool(name="h2", bufs=1) as h2_p, \
             tc.tile_pool(name="sz", bufs=1) as sz_p, \
             tc.tile_pool(name="w2", bufs=2) as w2_p, \
             tc.tile_pool(name="outp", bufs=1) as out_p:
            # res: x residual (f32), loaded here and updated in place.
            res = res_p.tile([128, 8 * IB], F32, tag="res", name="res")
            nc.sync.dma_start(res[:].rearrange("p (e i) -> p e i", e=8),
                              xr_d.rearrange("(e p) i -> p e i", p=128))
            resb = rb16_p.tile([128, 8 * IB], BF16, tag="rb16", name="rb16")
            h2 = h2_p.tile([128, 8 * IB], BF16, tag="h2", name="h2")
            sz = sz_p.tile([128, 32 * IB], BF16, tag="sz", name="sz")
            out_sb = out_p.tile([128, 8 * IB], F32, tag="out", name="out")
            w2h = []
            for k in range(4):
                t = w2_p.tile([128, 8 * D], BF16, tag="w2", name="w2")
                with tc.high_priority():
                    nc.scalar.dma_start(
                        t[:].rearrange("p (f e) -> p f e", f=8),
                        w2_d[k * 1024:(k + 1) * 1024, :]
                        .rearrange("(f p) e -> p f e", p=128))
                w2h.append(t)

            # ===== Phase D: out-projection + LN2 =======================
            with tc.tile_pool(name="sq2", bufs=3) as sq2_p, \
                 tc.tile_pool(name="lnr2", bufs=2) as lnr2_p, \
                 tc.tile_pool(name="lnb2", bufs=2) as lnb2_p:
                for et in range(8):
                    ps = ps_p.tile([128, IB], F32, tag="ps", name="psx1")
                    for dt in range(8):
                        nc.tensor.matmul(
                            ps[:],
                            wo[:, dt * D + et * 128: dt * D + (et + 1) * 128],
                            oT[:, dt * IB:(dt + 1) * IB],
                            start=(dt == 0), stop=(dt == 7))
                    rsl = res[:, et * IB:(et + 1) * IB]
                    nc.vector.scalar_tensor_tensor(
                        rsl, ps[:], boc[:, et:et + 1], rsl,
                        op0=OP.add, op1=OP.add)
                    nc.scalar.activation(resb[:, et * IB:(et + 1) * IB],
                                         rsl, AF.Copy)
                # LN2 stats (single 512-token block) on bf16 copy of res
                ps_mu = ps_p.tile([1, 512], F32, tag="ps", name="psmu2")
                for e in range(8):
                    nc.tensor.matmul(ps_mu[:], ones_b16[:],
                                     resb[:, e * IB:(e + 1) * IB],
                                     start=(e == 0), stop=(e == 7))
                ps_sq = ps_p.tile([1, 512], F32, tag="ps", name="pssq2")
                for e in range(8):
                    x2 = sq2_p.tile([128, 512], BF16, tag="x22", name="x22")
                    nc.scalar.square(x2[:], resb[:, e * IB:(e + 1) * IB])
                    nc.tensor.matmul(ps_sq[:], ones_b16[:], x2[:],
                                     start=(e == 0), stop=(e == 7))
                mu = lnr2_p.tile([1, 512], F32, tag="mu2r", name="mu2r")
                nc.scalar.activation(mu[:], ps_mu[:], AF.Identity, scale=1.0 / D)
                mu2 = lnr2_p.tile([1, 512], F32, tag="mu22", name="mu22")
                nc.scalar.square(mu2[:], mu[:])
                var = lnr2_p.tile([1, 512], F32, tag="var2", name="var2")
                nc.vector.scalar_tensor_tensor(var[:], ps_sq[:], 1.0 / D,
                                               mu2[:], op0=OP.mult,
                                               op1=OP.subtract)
                std = lnr2_p.tile([1, 512], F32, tag="std2", name="std2")
                nc.scalar.activation(std[:], var[:], AF.Sqrt, bias=eps_t[:])
                rstd = lnr2_p.tile([1, 512], F32, tag="rstd2", name="rstd2")
                nc.vector.reciprocal(rstd[:], std[:])
                mu_h = lnr2_p.tile([1, 512], BF16, tag="mu_h2", name="mu_h2")
                nc.scalar.activation(mu_h[:], mu[:], AF.Copy)
                rstd_h = lnr2_p.tile([1, 512], BF16, tag="rstd_h2",
                                     name="rstd_h2")
                nc.scalar.activation(rstd_h[:], rstd[:], AF.Copy)
                mu_b = lnb2_p.tile([128, 512], BF16, tag="mu_b2", name="mu_b2")
                nc.gpsimd.partition_broadcast(mu_b[:], mu_h[:])
                rstd_b = lnb2_p.tile([128, 512], BF16, tag="rstd_b2",
                                     name="rstd_b2")
                nc.gpsimd.partition_broadcast(rstd_b[:], rstd_h[:])
                for e in range(8):
                    hsl = h2[:, e * IB:(e + 1) * IB]
                    nc.vector.tensor_sub(hsl, resb[:, e * IB:(e + 1) * IB],
                                         mu_b[:])
                    nc.vector.tensor_mul(hsl, hsl, rstd_b[:])

            # ===== Phase E: FFN ========================================
            for w in range(8):
                pz = [ps_p.tile([128, IB], F32, tag="ps", name="psz")
                      for _ in range(4)]
                for e in range(8):
                    for fi in range(4):
                        ft = w * 4 + fi
                        k, fl = ft // 8, ft % 8
                        nc.tensor.matmul(
                            pz[fi][:],
                            w1h[k][:, e * 1024 + fl * 128:
                                   e * 1024 + (fl + 1) * 128],
                            h2[:, e * IB:(e + 1) * IB],
                            start=(e == 0), stop=(e == 7))
                for fi in range(4):
                    ft = w * 4 + fi
                    nc.scalar.activation(sz[:, ft * IB:(ft + 1) * IB],
                                         pz[fi][:], AF.Silu,
                                         bias=b1c[:, ft:ft + 1])
            ps_y = [ps_p.tile([128, IB], F32, tag="ps", name="psy")
                    for _ in range(8)]
            for ft in range(32):
                k2, fl2 = ft // 8, ft % 8
                for et in range(8):
                    nc.tensor.matmul(
                        ps_y[et][:],
                        w2h[k2][:, fl2 * D + et * 128: fl2 * D + (et + 1) * 128],
                        sz[:, ft * IB:(ft + 1) * IB],
                        start=(ft == 0), stop=(ft == 31))
            for et in range(8):
                nc.vector.scalar_tensor_tensor(
                    out_sb[:, et * IB:(et + 1) * IB], ps_y[et][:],
                    b2c[:, et:et + 1], res[:, et * IB:(et + 1) * IB],
                    op0=OP.add, op1=OP.add)
                if et % 4 == 3:
                    nc.sync.dma_start(
                        outT_d.rearrange("(e p) i -> p e i", p=128)
                        [:, et - 3:et + 1, :],
                        out_sb[:, (et - 3) * IB:(et + 1) * IB]
                        .rearrange("p (e i) -> p e i", e=4))

    nc.compile()
    return nc


def _prep_inputs(inputs):
    """Host-side layout prep -> list of 8 per-core input maps."""
    x = np.asarray(inputs["x"], dtype=np.float32)
    ab = np.asarray(inputs["attn_bias"], dtype=np.float32)
    g1 = np.asarray(inputs["ln1_g"], np.float32)
    b1n = np.asarray(inputs["ln1_b"], np.float32)
    g2 = np.asarray(inputs["ln2_g"], np.float32)
    b2n = np.asarray(inputs["ln2_b"], np.float32)
    Wq = np.asarray(inputs["Wq"], np.float32)
    Wk = np.asarray(inputs["Wk"], np.float32)
    Wv = np.asarray(inputs["Wv"], np.float32)
    Wo = np.asarray(inputs["Wo"], np.float32)
    W1 = np.asarray(inputs["W1"], np.float32)
    W2 = np.asarray(inputs["W2"], np.float32)

    def pack(v, ntiles):
        return np.ascontiguousarray(
            np.asarray(v, np.float32).reshape(ntiles, 128).T)

    shared = {
        "wq": np.ascontiguousarray((Wq * (g1[:, None] * SCALE)).astype(BF16_NP)),
        "wk": np.ascontiguousarray((Wk * g1[:, None]).astype(BF16_NP)),
        "wv": np.ascontiguousarray((Wv * g1[:, None]).astype(BF16_NP)),
        "wo": np.ascontiguousarray(Wo.astype(BF16_NP)),
        "w1": np.ascontiguousarray((W1 * g2[:, None]).astype(BF16_NP)),
        "w2": np.ascontiguousarray(W2.astype(BF16_NP)),
        "bqc": pack((np.asarray(inputs["bq"], np.float32) + b1n @ Wq) * SCALE, 8),
        "bkc": pack(np.asarray(inputs["bk"], np.float32) + b1n @ Wk, 8),
        "boc": pack(inputs["bo"], 8),
        "b1c": pack(np.asarray(inputs["b1"], np.float32) + b2n @ W1, 32),
        "b2c": pack(inputs["b2"], 8),
        "bvc": np.ascontiguousarray(
            (np.asarray(inputs["bv"], np.float32) + b1n @ Wv)
            .astype(BF16_NP).reshape(1, D)),
    }
    in_maps = []
    for c in range(N_CORES):
        b, i0 = c // 4, (c % 4) * IB
        xT = np.roll(x[b].T, -i0, axis=1)          # [D, T], queries at 0:IB
        ebf = np.exp(ab[b, :, i0:i0 + IB, :])      # [H, IB, T] f32
        eb = np.ascontiguousarray(
            np.roll(ebf, -i0, axis=2).transpose(0, 2, 1)).astype(BF16_NP)
        m = {
            "xb": np.ascontiguousarray(xT.astype(BF16_NP)),
            "xr": np.ascontiguousarray(xT[:, 0:IB]),
            "eb": eb,
        }
        m.update(shared)
        in_maps.append(m)
    return in_maps


def kernel(**inputs):
    if "nc" not in _cache:
        _cache["nc"] = build_program()
    nc = _cache["nc"]
    in_maps = _prep_inputs(inputs)
    r = run_bass_kernel_spmd(nc, in_maps, list(range(N_CORES)))
    out = np.empty((B, T, D), dtype=np.float32)
    for c in range(N_CORES):
        b, i0 = c // 4, (c % 4) * IB
        out[b, i0:i0 + IB, :] = np.asarray(r.results[c]["outT"], np.float32).T
    return out
